# revision 1
# baseline (speedup 1.0000x reference)
"""Trainium2 Bass kernel for ExcitationEmbedding + Ion RoPE.

Computes, for inputs
  excitations [256, 512, 2] int64 (pairs (a, b) with a, b in [0, 6)),
  n_electrons [256] f32, n_protons [256] f32,
  emb_weight  [26, 256] f32, lookup_table [6, 6] int64:

  idx   = lookup_table[a, b]                       # [B, N]
  emb   = emb_weight[idx]                          # [B, N, D]
  out   = per-batch block-diagonal rotation of emb (theta from n_electrons,
          phi from n_protons, 4-wide blocks: dims (0,1) by theta, (2,3) by phi)

Strategy (pure data parallel over 8 cores, 32 batches each):
  - flat code f = 6*a + b in [0, 36); one-hot against an iota constant.
  - emb36[j] = emb_weight[lut[j]] built once with a select-matmul (invalid
    j rows are 0). Per-batch rotated tables rot[j, b, d] are built with
    plain fp16 tensor ops on group tiles; the per-batch cos/sin patterns
    and flat codes are replicated across the 36 table partitions via
    DRAM->DRAM row staging + contiguous read-back (big DMA packets).
  - Gather is a PE fp16 matmul: chunk c of batch b covers tokens {4k+c};
    all 4 chunks land in one 2-bank PSUM tile, evacuated with a single
    [128, 1024] copy, and the per-batch 512 KB output DMA is fully linear.
  - Work is pipelined in 8 groups of 4 batches.
"""

import functools

import numpy as np

import concourse.bass as bass
import concourse.bacc as bacc
import concourse.mybir as mybir
from concourse import tile
from concourse.bass_utils import run_bass_kernel_spmd

B, N, D = 256, 512, 256
N_CORES = 8
BL = B // N_CORES   # 32 batches per core
G = 4               # batches per pipeline group
ANGLE_SCALE = 0.05
HALF_PI = float(np.pi / 2)

F32 = mybir.dt.float32
F16 = mybir.dt.float16
I16 = mybir.dt.int16
AF = mybir.ActivationFunctionType
ALU = mybir.AluOpType


def build_bass() -> bass.Bass:
    nc = bacc.Bacc(
        "TRN2", target_bir_lowering=False, debug=False, num_devices=N_CORES
    )

    exc = nc.dram_tensor("exc", [BL, N * 2], I16, kind="ExternalInput")
    ne = nc.dram_tensor("ne", [BL, 1], F32, kind="ExternalInput")
    npr = nc.dram_tensor("npr", [BL, 1], F32, kind="ExternalInput")
    emb = nc.dram_tensor("emb", [26, D], F32, kind="ExternalInput")
    lut = nc.dram_tensor("lut", [1, 36], F32, kind="ExternalInput")
    out = nc.dram_tensor("out", [BL, N, D], F32, kind="ExternalOutput")

    iota_f32 = nc.inline_tensor(
        np.arange(36, dtype=np.float32).reshape(36, 1), "iota_f32")

    with tile.TileContext(nc) as tc:
        with (
            tc.tile_pool(name="const", bufs=1) as const,
            tc.tile_pool(name="gpool", bufs=3) as gpool,
            tc.tile_pool(name="opool", bufs=4) as opool,
            tc.tile_pool(name="dram", bufs=1, space="DRAM") as dram,
            tc.tile_pool(name="psum_s", bufs=1, space="PSUM") as psum_s,
            tc.tile_pool(name="psum", bufs=3, space="PSUM") as psum,
        ):
            # ---- loads ----
            exc_s = const.tile([BL, N * 2], I16)
            nc.sync.dma_start(out=exc_s[:], in_=exc[:])
            ne_s = const.tile([BL, 1], F32)
            nc.sync.dma_start(out=ne_s[:], in_=ne[:])
            npr_s = const.tile([BL, 1], F32)
            nc.sync.dma_start(out=npr_s[:], in_=npr[:])
            emb_f = const.tile([26, D], F32)
            nc.sync.dma_start(out=emb_f[:], in_=emb[:])
            emb_s = const.tile([26, D], F16)
            nc.vector.tensor_copy(emb_s[:], emb_f[:])
            iota_s = const.tile([36, 1], F32)
            nc.sync.dma_start(out=iota_s[:], in_=iota_f32[:])

            # ---- flat codes: flat[b, n] = 6*a + b (fp16, values < 36) ----
            exc3 = exc_s[:].rearrange("q (n two) -> q n two", two=2)
            a_f = const.tile([BL, N], F32)
            nc.vector.tensor_copy(a_f[:], exc3[:, :, 0])
            b_f = const.tile([BL, N], F32)
            nc.vector.tensor_copy(b_f[:], exc3[:, :, 1])
            flat = const.tile([BL, N], F16)
            nc.vector.scalar_tensor_tensor(out=flat[:], in0=a_f[:], scalar=6.0,
                                           in1=b_f[:], op0=ALU.mult, op1=ALU.add)


            # ---- per-batch angle columns [BL, 1] ----
            hp = const.tile([BL, 1], F32)
            nc.vector.memset(hp[:], HALF_PI)
            # cos(t) = sin(pi/2 - t) keeps the LUT argument within [-pi, pi]
            ct = const.tile([BL, 1], F32)
            nc.scalar.activation(ct[:], ne_s[:], AF.Sin, bias=hp[:],
                                 scale=-ANGLE_SCALE)
            st = const.tile([BL, 1], F32)
            nc.scalar.activation(st[:], ne_s[:], AF.Sin, bias=0.0, scale=ANGLE_SCALE)
            nst = const.tile([BL, 1], F32)
            nc.scalar.activation(nst[:], ne_s[:], AF.Sin, bias=0.0, scale=-ANGLE_SCALE)
            cp = const.tile([BL, 1], F32)
            nc.scalar.activation(cp[:], npr_s[:], AF.Sin, bias=hp[:],
                                 scale=-ANGLE_SCALE)
            sp = const.tile([BL, 1], F32)
            nc.scalar.activation(sp[:], npr_s[:], AF.Sin, bias=0.0, scale=ANGLE_SCALE)
            nsp = const.tile([BL, 1], F32)
            nc.scalar.activation(nsp[:], npr_s[:], AF.Sin, bias=0.0, scale=-ANGLE_SCALE)

            # natural layout: C_all[b, 4k+i] = (ct,ct,cp,cp)[i],
            #                 S_all[b, 4k+i] = (st,-st,sp,-sp)[i]
            ones = const.tile([BL, 64], F16)
            nc.vector.memset(ones[:], 1.0)
            c_all = const.tile([BL, D], F16)
            s_all = const.tile([BL, D], F16)
            c4 = c_all[:].rearrange("q (k i) -> q k i", i=4)
            s4 = s_all[:].rearrange("q (k i) -> q k i", i=4)
            for i, col in enumerate([ct, ct, cp, cp]):
                nc.vector.tensor_scalar(out=c4[:, :, i], in0=ones[:],
                                        scalar1=col[:], scalar2=None, op0=ALU.mult)
            for i, col in enumerate([st, nst, sp, nsp]):
                nc.vector.tensor_scalar(out=s4[:, :, i], in0=ones[:],
                                        scalar1=col[:], scalar2=None, op0=ALU.mult)

            # ---- select matrix: selT[r, j] = (lut_flat[j] == r), r in [0,26) ----
            lut_bc = const.tile([26, 36], F32)
            nc.sync.dma_start(out=lut_bc[:], in_=lut[0:1, :].to_broadcast((26, 36)))
            selT = const.tile([26, 36], F16)
            nc.vector.tensor_scalar(out=selT[:], in0=lut_bc[:],
                                    scalar1=iota_s[0:26, :], scalar2=None,
                                    op0=ALU.is_equal)

            # ---- 36-row gathered table (natural layout) + pair-swapped copy
            eph_ps = psum_s.tile([36, D], F32)
            nc.tensor.matmul(eph_ps[:], selT[:], emb_s[:], start=True, stop=True)
            e_ph = const.tile([36, D], F16)
            nc.scalar.activation(e_ph[:], eph_ps[:], AF.Copy)
            e_sw = const.tile([36, D], F16)
            ep2 = e_ph[:].rearrange("j (k i) -> j k i", i=2)
            es2 = e_sw[:].rearrange("j (k i) -> j k i", i=2)
            nc.vector.tensor_copy(es2[:, :, 0], ep2[:, :, 1])
            nc.vector.tensor_copy(es2[:, :, 1], ep2[:, :, 0])
            # group-width replicas (4 copies of the table along free dim)
            emb_t4 = const.tile([36, G, D], F16)
            emb_sw4 = const.tile([36, G, D], F16)
            for i in range(G):
                nc.vector.tensor_copy(emb_t4[:, i, :], e_ph[:])
                nc.vector.tensor_copy(emb_sw4[:, i, :], e_sw[:])

            # ---- DRAM bounce; input-side DMAs ride SWDGE so the sync
            # engine's HWDGE FIFO carries only output writes ----
            flat_d = dram.tile([BL, N], F16)
            nc.sync.dma_start(out=flat_d[:], in_=flat[:])
            c_all_d = dram.tile([BL, D], F16)
            nc.sync.dma_start(out=c_all_d[:], in_=c_all[:])
            s_all_d = dram.tile([BL, D], F16)
            nc.sync.dma_start(out=s_all_d[:], in_=s_all[:])

            flat_big = const.tile([36, BL, N], F16)
            rot_big = const.tile([36, BL, D], F16)
            flat_flat = flat_d[:].rearrange("q n -> (q n)")
            c_flat = c_all_d[:].rearrange("q d -> (q d)")
            s_flat = s_all_d[:].rearrange("q d -> (q d)")

            group_sizes = [G] * (BL // G)
            group_starts = [sum(group_sizes[:i]) for i in range(len(group_sizes))]
            for g, (g0, gn) in enumerate(zip(group_starts, group_sizes)):
                gs = slice(g0, g0 + gn)
                # broadcast-read of this group's rows: contiguous inner dim
                # -> 36 fat packets per transfer
                dmae = nc.sync if g < 2 else nc.gpsimd
                dmae.dma_start(
                    out=flat_big[:, gs, :],
                    in_=flat_flat[g0 * N:(g0 + gn) * N].unsqueeze(0)
                    .to_broadcast((36, gn * N)))
                # one-hot in place
                nc.vector.tensor_scalar(out=flat_big[:, gs, :],
                                        in0=flat_big[:, gs, :],
                                        scalar1=iota_s[:], scalar2=None,
                                        op0=ALU.is_equal)
                cbg = gpool.tile([36, gn, D], F16, tag="cbg", bufs=3)
                dmae.dma_start(
                    out=cbg[:],
                    in_=c_flat[g0 * D:(g0 + gn) * D].unsqueeze(0)
                    .to_broadcast((36, gn * D)))
                sbg = gpool.tile([36, gn, D], F16, tag="sbg", bufs=3)
                dmae.dma_start(
                    out=sbg[:],
                    in_=s_flat[g0 * D:(g0 + gn) * D].unsqueeze(0)
                    .to_broadcast((36, gn * D)))
                t1g = gpool.tile([36, gn, D], F16, tag="t1g", bufs=3)
                nc.vector.tensor_mul(t1g[:], emb_t4[:, :gn, :], cbg[:])
                t2g = gpool.tile([36, gn, D], F16, tag="t2g", bufs=3)
                nc.vector.tensor_mul(t2g[:], emb_sw4[:, :gn, :], sbg[:])
                nc.vector.tensor_add(rot_big[:, gs, :], t1g[:], t2g[:])

                for b in range(g0, g0 + gn):
                    ps = psum.tile([128, 4 * D], F32)
                    for c in range(4):
                        # chunk c covers tokens {4k + c}
                        nc.tensor.matmul(ps[:, c * D:(c + 1) * D],
                                         flat_big[:, b, c::4], rot_big[:, b, :],
                                         start=True, stop=True)
                    obuf = opool.tile([128, 4 * D], F32)
                    nc.vector.tensor_copy(obuf[:, 0:D], ps[:, 0:D])
                    nc.scalar.activation(obuf[:, D:], ps[:, D:], AF.Copy)
                    # token t = 4k + c sits at obuf[k, c*256:(c+1)*256] ->
                    # this DRAM view is fully linear (contiguous 512 KB write)
                    nc.sync.dma_start(
                        out=out[b].rearrange("(p c) d -> p c d", p=128),
                        in_=obuf[:])

    nc.compile()
    return nc


@functools.lru_cache(maxsize=1)
def _get_nc() -> bass.Bass:
    return build_bass()


def kernel_with_results(excitations, n_electrons, n_protons, emb_weight,
                        lookup_table, trace=False):
    exc = np.asarray(excitations)
    exc16 = exc.astype(np.int16).reshape(B, N * 2)
    ne = np.ascontiguousarray(np.asarray(n_electrons, dtype=np.float32))
    npr = np.ascontiguousarray(np.asarray(n_protons, dtype=np.float32))
    emb = np.ascontiguousarray(np.asarray(emb_weight, dtype=np.float32))
    lut_f = np.asarray(lookup_table).astype(np.float32).reshape(1, 36)
    lut_f = np.ascontiguousarray(lut_f)

    in_maps = []
    for c in range(N_CORES):
        sl = slice(c * BL, (c + 1) * BL)
        in_maps.append({
            "exc": np.ascontiguousarray(exc16[sl]),
            "ne": np.ascontiguousarray(ne[sl].reshape(BL, 1)),
            "npr": np.ascontiguousarray(npr[sl].reshape(BL, 1)),
            "emb": emb,
            "lut": lut_f,
        })

    nc = _get_nc()
    res = run_bass_kernel_spmd(nc, in_maps, list(range(N_CORES)), trace=trace)
    out_arr = np.concatenate(
        [res.results[c]["out"] for c in range(N_CORES)], axis=0)
    return np.ascontiguousarray(out_arr.reshape(B, N, D).astype(np.float32)), res


def kernel(excitations, n_electrons, n_protons, emb_weight, lookup_table):
    out_arr, _ = kernel_with_results(excitations, n_electrons, n_protons,
                                     emb_weight, lookup_table)
    return out_arr



# revision 3
# speedup vs baseline: 1.1746x; 1.1746x over previous
"""Trainium2 Bass kernel for ExcitationEmbedding + Ion RoPE.

Computes, for inputs
  excitations [256, 512, 2] int64 (pairs (a, b) with a, b in [0, 6)),
  n_electrons [256] f32, n_protons [256] f32,
  emb_weight  [26, 256] f32, lookup_table [6, 6] int64:

  idx   = lookup_table[a, b]                       # [B, N]
  emb   = emb_weight[idx]                          # [B, N, D]
  out   = per-batch block-diagonal rotation of emb (theta from n_electrons,
          phi from n_protons, 4-wide blocks: dims (0,1) by theta, (2,3) by phi)

Strategy (pure data parallel over 8 cores, 32 batches each):
  - Host packs each excitation pair into one int16 code a + 256*b and
    inverts lookup_table into a 26-entry per-row code list, so the device
    gather is a one-hot match against the embedding rows directly.
  - All 26-partition work is packed x4 into 104 partitions (4 blocks of
    8 batches): one is_equal builds every one-hot, and the per-batch
    rotated tables rot[j, b, d] = e[j,d]*c(b,d) + e_sw[j,d]*s(b,d) are
    2 muls (free-dim-broadcast embedding operand) + 1 add.
  - Gather is a PE fp16 matmul per (batch, token-chunk); chunk c of
    batch b covers tokens {4k+c}; all 4 chunks of a batch land in one
    2-bank PSUM tile, evacuated f32->fp16 by the scalar engine (fast
    PSUM path) straight into a 4-batch group buffer.
  - Output is fp16 (rel-err budget 2e-2, fp16 costs ~3e-4): halves HBM
    write traffic. One 1 MB linear DMA per 4-batch group on the sync
    queue; all input-side DMAs ride the scalar HWDGE queue.
"""

import functools

import numpy as np

import concourse.bass as bass
import concourse.bacc as bacc
import concourse.mybir as mybir
from concourse import tile
from concourse.bass_utils import run_bass_kernel_spmd

B, N, D = 256, 512, 256
N_CORES = 8
BL = B // N_CORES   # 32 batches per core
ANGLE_SCALE = 0.05
HALF_PI = float(np.pi / 2)

F32 = mybir.dt.float32
F16 = mybir.dt.float16
I16 = mybir.dt.int16
AF = mybir.ActivationFunctionType
ALU = mybir.AluOpType


def build_bass() -> bass.Bass:
    nc = bacc.Bacc(
        "TRN2", target_bir_lowering=False, debug=False, num_devices=N_CORES
    )

    exc = nc.dram_tensor("exc", [BL, N], I16, kind="ExternalInput")  # a + 256*b
    ne = nc.dram_tensor("ne", [BL, 1], F32, kind="ExternalInput")
    npr = nc.dram_tensor("npr", [BL, 1], F32, kind="ExternalInput")
    emb = nc.dram_tensor("emb", [26, D], F32, kind="ExternalInput")
    codes = nc.dram_tensor("codes", [128, 1], F32, kind="ExternalInput")
    out = nc.dram_tensor("out", [BL, N, D], F16, kind="ExternalOutput")

    exc_flat = exc[:].rearrange("q n -> (q n)")

    with tile.TileContext(nc) as tc:
        with (
            tc.tile_pool(name="const", bufs=1) as const,
            tc.tile_pool(name="opool", bufs=2) as opool,
            tc.tile_pool(name="dram", bufs=1, space="DRAM") as dram,
            tc.tile_pool(name="psum", bufs=3, space="PSUM") as psum,
        ):
            # ---- input loads (sync queue; all early) ----
            ne_s = const.tile([BL, 1], F32)
            nc.sync.dma_start(out=ne_s[:], in_=ne[:])
            npr_s = const.tile([BL, 1], F32)
            nc.sync.dma_start(out=npr_s[:], in_=npr[:])
            codes_s = const.tile([128, 1], F32)
            nc.sync.dma_start(out=codes_s[:], in_=codes[:])

            # ---- packed-code broadcast: block q holds batches 8q..8q+7 ----
            exc_bc = const.tile([128, 8, N], I16)
            for q in range(4):
                nc.scalar.dma_start(
                    out=exc_bc[32 * q:32 * q + 26, :, :],
                    in_=exc_flat[q * 8 * N:(q + 1) * 8 * N].unsqueeze(0)
                    .to_broadcast((26, 8 * N)))

            # ---- per-batch angle columns [BL, 1] ----
            hp = const.tile([BL, 1], F32)
            nc.vector.memset(hp[:], HALF_PI)
            # cos(t) = sin(pi/2 - t) keeps the LUT argument within [-pi, pi]
            ct = const.tile([BL, 1], F32)
            nc.scalar.activation(ct[:], ne_s[:], AF.Sin, bias=hp[:],
                                 scale=-ANGLE_SCALE)
            st = const.tile([BL, 1], F32)
            nc.scalar.activation(st[:], ne_s[:], AF.Sin, bias=0.0, scale=ANGLE_SCALE)
            nst = const.tile([BL, 1], F32)
            nc.scalar.activation(nst[:], ne_s[:], AF.Sin, bias=0.0, scale=-ANGLE_SCALE)
            cp = const.tile([BL, 1], F32)
            nc.scalar.activation(cp[:], npr_s[:], AF.Sin, bias=hp[:],
                                 scale=-ANGLE_SCALE)
            sp = const.tile([BL, 1], F32)
            nc.scalar.activation(sp[:], npr_s[:], AF.Sin, bias=0.0, scale=ANGLE_SCALE)
            nsp = const.tile([BL, 1], F32)
            nc.scalar.activation(nsp[:], npr_s[:], AF.Sin, bias=0.0, scale=-ANGLE_SCALE)

            # natural layout: c_all[b, 4k+i] = (ct,ct,cp,cp)[i],
            #                 s_all[b, 4k+i] = (st,-st,sp,-sp)[i]
            ones = const.tile([BL, 64], F16)
            nc.vector.memset(ones[:], 1.0)
            c_all = const.tile([BL, D], F16)
            s_all = const.tile([BL, D], F16)
            c4 = c_all[:].rearrange("q (k i) -> q k i", i=4)
            s4 = s_all[:].rearrange("q (k i) -> q k i", i=4)
            for i, col in enumerate([ct, ct, cp, cp]):
                nc.vector.tensor_scalar(out=c4[:, :, i], in0=ones[:],
                                        scalar1=col[:], scalar2=None, op0=ALU.mult)
            for i, col in enumerate([st, nst, sp, nsp]):
                nc.vector.tensor_scalar(out=s4[:, :, i], in0=ones[:],
                                        scalar1=col[:], scalar2=None, op0=ALU.mult)

            # ---- DRAM bounce for partition-broadcast of c/s patterns ----
            c_d = dram.tile([BL, D], F16)
            nc.sync.dma_start(out=c_d[:], in_=c_all[:])
            s_d = dram.tile([BL, D], F16)
            nc.sync.dma_start(out=s_d[:], in_=s_all[:])
            c_flat = c_d[:].rearrange("q d -> (q d)")
            s_flat = s_d[:].rearrange("q d -> (q d)")

            c_bc = const.tile([128, 8, D], F16)
            s_bc = const.tile([128, 8, D], F16)
            for q in range(4):
                nc.scalar.dma_start(
                    out=c_bc[32 * q:32 * q + 26, :, :],
                    in_=c_flat[q * 8 * D:(q + 1) * 8 * D].unsqueeze(0)
                    .to_broadcast((26, 8 * D)))
                nc.scalar.dma_start(
                    out=s_bc[32 * q:32 * q + 26, :, :],
                    in_=s_flat[q * 8 * D:(q + 1) * 8 * D].unsqueeze(0)
                    .to_broadcast((26, 8 * D)))

            # ---- embedding rows on all 4 blocks: eA plain, eB pair-swapped
            emb4_f = const.tile([128, D], F32)
            for q in range(4):
                nc.sync.dma_start(out=emb4_f[32 * q:32 * q + 26, :], in_=emb[:])
            eA = const.tile([128, D], F16)
            nc.vector.tensor_copy(eA[:], emb4_f[:])
            eB = const.tile([128, D], F16)
            eA2 = eA[:].rearrange("p (k i) -> p k i", i=2)
            eB2 = eB[:].rearrange("p (k i) -> p k i", i=2)
            nc.vector.tensor_copy(eB2[:, :, 0], eA2[:, :, 1])
            nc.vector.tensor_copy(eB2[:, :, 1], eA2[:, :, 0])

            # ---- one-hot + rotated tables, split in two halves for pipelining
            oh = const.tile([128, 8, N], F16)
            t12a = const.tile([128, 8, D], F16)
            t12b = const.tile([128, 8, D], F16)
            rot = const.tile([128, 8, D], F16)
            eAbc = eA[:].unsqueeze(1).to_broadcast((128, 4, D))
            eBbc = eB[:].unsqueeze(1).to_broadcast((128, 4, D))
            for s in range(2):
                hs = slice(4 * s, 4 * s + 4)
                nc.vector.tensor_scalar(out=oh[:, hs, :], in0=exc_bc[:, hs, :],
                                        scalar1=codes_s[:], scalar2=None,
                                        op0=ALU.is_equal)
                nc.vector.tensor_mul(t12a[:, hs, :], eAbc, c_bc[:, hs, :])
                nc.vector.tensor_mul(t12b[:, hs, :], eBbc, s_bc[:, hs, :])
                nc.vector.tensor_add(rot[:, hs, :], t12a[:, hs, :],
                                     t12b[:, hs, :])

            # ---- gather matmuls + evacuation + output, 4-batch groups ----
            for g in (0, 2, 4, 6, 1, 3, 5, 7):
                obuf = opool.tile([128, 4, 4 * D], F16, tag="obuf", bufs=2)
                for r in range(4):
                    b = 4 * g + r
                    q, rr = b // 8, b % 8
                    js = slice(32 * q, 32 * q + 26)
                    ps = psum.tile([128, 4 * D], F32, tag="ps", bufs=3)
                    for c in range(4):
                        # chunk c covers tokens {4k + c}
                        nc.tensor.matmul(ps[:, c * D:(c + 1) * D],
                                         oh[js, rr, c::4], rot[js, rr, :],
                                         start=True, stop=True,
                                         tile_position=(32 * q, 0))
                    nc.scalar.activation(obuf[:, r, :], ps[:], AF.Copy)
                # token t = 4k + c of batch 4g+r sits at obuf[k, r, c*256+d]
                # -> per-partition 4 linear 2 KB blocks, 1 MB total
                nc.sync.dma_start(
                    out=out[4 * g:4 * g + 4].rearrange(
                        "b (p c) d -> p b (c d)", p=128),
                    in_=obuf[:])

    nc.compile()
    return nc


@functools.lru_cache(maxsize=1)
def _get_nc() -> bass.Bass:
    return build_bass()


def kernel_with_results(excitations, n_electrons, n_protons, emb_weight,
                        lookup_table, trace=False):
    exc = np.asarray(excitations)
    codes16 = (exc[..., 0] + 256 * exc[..., 1]).astype(np.int16)  # [B, N]
    ne = np.ascontiguousarray(np.asarray(n_electrons, dtype=np.float32))
    npr = np.ascontiguousarray(np.asarray(n_protons, dtype=np.float32))
    emb = np.ascontiguousarray(np.asarray(emb_weight, dtype=np.float32))
    lut = np.asarray(lookup_table)
    codes32 = np.full((32,), 1e9, dtype=np.float32)  # pad: never matches
    for x in range(6):
        for y in range(6):
            r = int(lut[x, y])
            if 0 <= r < 26:
                codes32[r] = float(x + 256 * y)
    codes128 = np.ascontiguousarray(np.tile(codes32, 4).reshape(128, 1))

    in_maps = []
    for c in range(N_CORES):
        sl = slice(c * BL, (c + 1) * BL)
        in_maps.append({
            "exc": np.ascontiguousarray(codes16[sl]),
            "ne": np.ascontiguousarray(ne[sl].reshape(BL, 1)),
            "npr": np.ascontiguousarray(npr[sl].reshape(BL, 1)),
            "emb": emb,
            "codes": codes128,
        })

    nc = _get_nc()
    res = run_bass_kernel_spmd(nc, in_maps, list(range(N_CORES)), trace=trace)
    out_arr = np.concatenate(
        [res.results[c]["out"] for c in range(N_CORES)], axis=0)
    return np.ascontiguousarray(out_arr.reshape(B, N, D).astype(np.float32)), res


def kernel(excitations, n_electrons, n_protons, emb_weight, lookup_table):
    out_arr, _ = kernel_with_results(excitations, n_electrons, n_protons,
                                     emb_weight, lookup_table)
    return out_arr


# revision 4
# speedup vs baseline: 1.2934x; 1.1012x over previous
"""Trainium2 Bass kernel for ExcitationEmbedding + Ion RoPE.

Computes, for inputs
  excitations [256, 512, 2] int64 (pairs (a, b) with a, b in [0, 6)),
  n_electrons [256] f32, n_protons [256] f32,
  emb_weight  [26, 256] f32, lookup_table [6, 6] int64:

  idx   = lookup_table[a, b]                       # [B, N]
  emb   = emb_weight[idx]                          # [B, N, D]
  out   = per-batch block-diagonal rotation of emb (theta from n_electrons,
          phi from n_protons, 4-wide blocks: dims (0,1) by theta, (2,3) by phi)

Strategy (pure data parallel over 8 cores, 32 batches each):
  - Host packs each excitation pair into one int8 code a + 16*b and inverts
    lookup_table into a per-row code list, so the device gather is a single
    is_equal against the embedding rows (no on-device index math, no
    DRAM bounce for the codes).
  - 26-row work is packed x4 onto partitions {0,32,64,96} (4 blocks of
    8 batches; matmuls pass explicit tile_position).  Per-batch rotated
    tables rot[j,b,d] = e[j,d]*c(b,d) + e_sw[j,d]*s(b,d) come from a tiny
    [32, 8] angle-pattern tile (the only DRAM bounce, 512 B) broadcast to
    all blocks, with both table muls using free-dim-broadcast APs.
  - Gather is a PE fp16 matmul per (batch, token-chunk); chunk c covers
    tokens {4k+c}.  Two batches share one 4-bank PSUM tile; evacuation
    (f32 -> fp16 cast) is split between the scalar and vector engines by
    whole pairs to avoid PSUM bank sharing.  A burst of dummy matmuls at
    t=0 keeps the PE busy so the HAM clock gate is warm (2.4 GHz) when
    the real matmuls start.
  - Output is fp16 (the one-hot gather output is exactly fp16): halves
    HBM write traffic vs f32.  One 1 MB linear DMA per 4-batch group on
    the sync queue; broadcasts ride the gpsimd (SWDGE) queue.
"""

import functools

import numpy as np

import concourse.bass as bass
import concourse.bacc as bacc
import concourse.mybir as mybir
from concourse import tile
from concourse.bass_utils import run_bass_kernel_spmd

B, N, D = 256, 512, 256
N_CORES = 8
BL = B // N_CORES   # 32 batches per core
ANGLE_SCALE = 0.05
HALF_PI = float(np.pi / 2)
N_WARM = 34         # dummy matmuls bridging startup so the PE HAM stays busy

F32 = mybir.dt.float32
F16 = mybir.dt.float16
I8 = mybir.dt.int8
AF = mybir.ActivationFunctionType
ALU = mybir.AluOpType


def build_bass() -> bass.Bass:
    nc = bacc.Bacc(
        "TRN2", target_bir_lowering=False, debug=False, num_devices=N_CORES
    )

    exc = nc.dram_tensor("exc", [BL, N], I8, kind="ExternalInput")  # a + 16*b
    ne = nc.dram_tensor("ne", [BL, 1], F32, kind="ExternalInput")
    npr = nc.dram_tensor("npr", [BL, 1], F32, kind="ExternalInput")
    emb = nc.dram_tensor("emb", [26, D], F32, kind="ExternalInput")
    codes = nc.dram_tensor("codes", [128, 1], F32, kind="ExternalInput")
    out = nc.dram_tensor("out", [BL, N, D], F16, kind="ExternalOutput")

    exc_flat = exc[:].rearrange("q n -> (q n)")

    with tile.TileContext(nc) as tc:
        with (
            tc.tile_pool(name="const", bufs=1) as const,
            tc.tile_pool(name="opool", bufs=2) as opool,
            tc.tile_pool(name="dram", bufs=1, space="DRAM") as dram,
            tc.tile_pool(name="psum", bufs=2, space="PSUM") as psum,
        ):
            # ---- warmup operands + first PSUM pair tile (dummy target) ----
            warm_w = const.tile([32, 128], F16)
            nc.vector.memset(warm_w[:], 0.0)
            warm_x = const.tile([32, D], F16)
            nc.vector.memset(warm_x[:], 0.0)
            ps0 = psum.tile([128, 2 * 4 * D], F32, tag="ps", bufs=2)
            for _ in range(N_WARM):
                nc.tensor.matmul(ps0[:, 0:D], warm_w[:], warm_x[:],
                                 start=True, stop=True)

            # ---- input loads (sync queue; all early) ----
            ne_s = const.tile([BL, 1], F32)
            nc.sync.dma_start(out=ne_s[:], in_=ne[:])
            npr_s = const.tile([BL, 1], F32)
            nc.sync.dma_start(out=npr_s[:], in_=npr[:])
            codes_s = const.tile([128, 1], F32)
            nc.sync.dma_start(out=codes_s[:], in_=codes[:])
            emb4_f = const.tile([128, D], F32)
            for q in range(4):
                nc.sync.dma_start(out=emb4_f[32 * q:32 * q + 26, :], in_=emb[:])

            # ---- packed-code broadcast: block q holds batches 8q..8q+7 ----
            exc_bc = const.tile([128, 8, N], I8)
            for q in range(4):
                nc.gpsimd.dma_start(
                    out=exc_bc[32 * q:32 * q + 26, :, :],
                    in_=exc_flat[q * 8 * N:(q + 1) * 8 * N].unsqueeze(0)
                    .to_broadcast((26, 8 * N)))

            # ---- angles: 6 sins, then the 8-wide per-batch pattern tile
            # pattern = (ct, ct, cp, cp, st, -st, sp, -sp)
            hp = const.tile([BL, 1], F32)
            nc.vector.memset(hp[:], HALF_PI)
            # cos(t) = sin(pi/2 - t) keeps the LUT argument within [-pi, pi]
            ct = const.tile([BL, 1], F32)
            nc.scalar.activation(ct[:], ne_s[:], AF.Sin, bias=hp[:],
                                 scale=-ANGLE_SCALE)
            st = const.tile([BL, 1], F32)
            nc.scalar.activation(st[:], ne_s[:], AF.Sin, bias=0.0, scale=ANGLE_SCALE)
            nst = const.tile([BL, 1], F32)
            nc.scalar.activation(nst[:], ne_s[:], AF.Sin, bias=0.0, scale=-ANGLE_SCALE)
            cp = const.tile([BL, 1], F32)
            nc.scalar.activation(cp[:], npr_s[:], AF.Sin, bias=hp[:],
                                 scale=-ANGLE_SCALE)
            sp = const.tile([BL, 1], F32)
            nc.scalar.activation(sp[:], npr_s[:], AF.Sin, bias=0.0, scale=ANGLE_SCALE)
            nsp = const.tile([BL, 1], F32)
            nc.scalar.activation(nsp[:], npr_s[:], AF.Sin, bias=0.0, scale=-ANGLE_SCALE)

            ang_st = const.tile([BL, 8], F16)
            for i, col in enumerate([ct, ct, cp, cp, st, nst, sp, nsp]):
                nc.vector.tensor_copy(ang_st[:, i:i + 1], col[:])
            ang_d = dram.tile([BL, 8], F16)
            nc.sync.dma_start(out=ang_d[:], in_=ang_st[:])
            ang_flat = ang_d[:].rearrange("q i -> (q i)")
            ang_bc = const.tile([128, 8, 8], F16)
            for q in range(4):
                nc.gpsimd.dma_start(
                    out=ang_bc[32 * q:32 * q + 26, :, :],
                    in_=ang_flat[q * 64:(q + 1) * 64].unsqueeze(0)
                    .to_broadcast((26, 64)))

            # ---- embedding rows on all 4 blocks: eA plain, eB pair-swapped
            eA = const.tile([128, D], F16)
            nc.vector.tensor_copy(eA[:], emb4_f[:])
            eB = const.tile([128, D], F16)
            eA2 = eA[:].rearrange("p (k i) -> p k i", i=2)
            eB2 = eB[:].rearrange("p (k i) -> p k i", i=2)
            nc.vector.tensor_copy(eB2[:, :, 0], eA2[:, :, 1])
            nc.vector.tensor_copy(eB2[:, :, 1], eA2[:, :, 0])

            # ---- one-hot + rotated tables, in two halves for pipelining ----
            oh = const.tile([128, 8, N], F16)
            t12a = const.tile([128, 8, 64, 4], F16)
            t12b = const.tile([128, 8, 64, 4], F16)
            rot = const.tile([128, 8, D], F16)
            rot4 = rot[:].rearrange("p r (k i) -> p r k i", i=4)
            eA4 = eA[:].rearrange("p (k i) -> p k i", i=4)
            eB4 = eB[:].rearrange("p (k i) -> p k i", i=4)
            for s in range(2):
                hs = slice(4 * s, 4 * s + 4)
                nc.vector.tensor_scalar(out=oh[:, hs, :], in0=exc_bc[:, hs, :],
                                        scalar1=codes_s[:], scalar2=None,
                                        op0=ALU.is_equal)
                nc.vector.tensor_mul(
                    t12a[:, hs, :, :],
                    eA4.unsqueeze(1).to_broadcast((128, 4, 64, 4)),
                    ang_bc[:, hs, 0:4].unsqueeze(2).to_broadcast((128, 4, 64, 4)))
                nc.vector.tensor_mul(
                    t12b[:, hs, :, :],
                    eB4.unsqueeze(1).to_broadcast((128, 4, 64, 4)),
                    ang_bc[:, hs, 4:8].unsqueeze(2).to_broadcast((128, 4, 64, 4)))
                nc.vector.tensor_add(rot4[:, hs, :, :], t12a[:, hs, :, :],
                                     t12b[:, hs, :, :])

            # ---- gather matmuls + paired evacuation + output ----
            pair_idx = 0
            for g in (0, 2, 4, 6, 1, 3, 5, 7):
                obuf = opool.tile([128, 4, 4 * D], F16, tag="obuf", bufs=2)
                for j in range(2):
                    ps = ps0 if pair_idx == 0 else psum.tile(
                        [128, 2 * 4 * D], F32, tag="ps", bufs=2)
                    for jj in range(2):
                        b = 4 * g + 2 * j + jj
                        q, rr = b // 8, b % 8
                        js = slice(32 * q, 32 * q + 26)
                        for c in range(4):
                            # chunk c covers tokens {4k + c}
                            nc.tensor.matmul(
                                ps[:, (4 * jj + c) * D:(4 * jj + c + 1) * D],
                                oh[js, rr, c::4], rot[js, rr, :],
                                start=True, stop=True,
                                tile_position=(32 * q, 0))
                    ps2 = ps[:].rearrange("p (b f) -> p b f", b=2)
                    ob2 = obuf[:, 2 * j:2 * j + 2, :]
                    if pair_idx % 3 == 2 or pair_idx == 15:
                        nc.vector.tensor_copy(ob2, ps2)
                    else:
                        nc.scalar.activation(ob2, ps2, AF.Copy)
                    pair_idx += 1
                # token t = 4k + c of batch 4g+r sits at obuf[k, r, c*256+d]
                # -> per-partition 4 linear 2 KB blocks, 1 MB total
                nc.sync.dma_start(
                    out=out[4 * g:4 * g + 4].rearrange(
                        "b (p c) d -> p b (c d)", p=128),
                    in_=obuf[:])

    nc.compile()
    return nc


@functools.lru_cache(maxsize=1)
def _get_nc() -> bass.Bass:
    return build_bass()


def kernel_with_results(excitations, n_electrons, n_protons, emb_weight,
                        lookup_table, trace=False):
    exc = np.asarray(excitations)
    codes8 = (exc[..., 0] + 16 * exc[..., 1]).astype(np.int8)  # [B, N]
    ne = np.ascontiguousarray(np.asarray(n_electrons, dtype=np.float32))
    npr = np.ascontiguousarray(np.asarray(n_protons, dtype=np.float32))
    emb = np.ascontiguousarray(np.asarray(emb_weight, dtype=np.float32))
    lut = np.asarray(lookup_table)
    codes32 = np.full((32,), 1e9, dtype=np.float32)  # pad: never matches
    for x in range(6):
        for y in range(6):
            r = int(lut[x, y])
            if 0 <= r < 26:
                codes32[r] = float(x + 16 * y)
    codes128 = np.ascontiguousarray(np.tile(codes32, 4).reshape(128, 1))

    in_maps = []
    for c in range(N_CORES):
        sl = slice(c * BL, (c + 1) * BL)
        in_maps.append({
            "exc": np.ascontiguousarray(codes8[sl]),
            "ne": np.ascontiguousarray(ne[sl].reshape(BL, 1)),
            "npr": np.ascontiguousarray(npr[sl].reshape(BL, 1)),
            "emb": emb,
            "codes": codes128,
        })

    nc = _get_nc()
    res = run_bass_kernel_spmd(nc, in_maps, list(range(N_CORES)), trace=trace)
    out_arr = np.concatenate(
        [res.results[c]["out"] for c in range(N_CORES)], axis=0)
    return np.ascontiguousarray(out_arr.reshape(B, N, D).astype(np.float32)), res


def kernel(excitations, n_electrons, n_protons, emb_weight, lookup_table):
    out_arr, _ = kernel_with_results(excitations, n_electrons, n_protons,
                                     emb_weight, lookup_table)
    return out_arr
